# revision 1
# baseline (speedup 1.0000x reference)
"""TRN2 Bass kernel for 2-layer GAT + grouped softmax (nn_Actor_1881195675935).

8-core SPMD, nodes sharded contiguously (12500/core, padded to 12544 = 98
tiles of 128); edges live with the owner of their dst node in an ELLPACK
layout (partition = dst node, free axis = edge slots).

v2 design (vs v1): the gather table stores TWO nodes per 256B row (64 bf16
each: [h | a_src | pad]), so the int16 gather indices cover the whole table
with only 2 windows and ~28% fewer descriptors than the 4-window f32 layout.
Which half of a fetched pair-row belongs to a slot is resolved with
host-baked additive penalty masks (0 / -1e4) applied to BOTH halves' a_src
logits; exp() of the poisoned half underflows to exactly 0, so both halves
can always be accumulated. prelu+exp run on the Scalar (ACT) engine over one
contiguous slot axis; DVE does a handful of large fused bf16 ops per tile.
AllGathers move the packed bf16 table (12.85MB vs 25.7MB f32). The final
`index`-grouped softmax keeps the v1 scheme: baked fp8 one-hot matmuls + a
64KB AllReduce.
"""

import sys

sys.path.insert(0, "/opt/trn_rl_repo")

import numpy as np
import ml_dtypes  # noqa: F401

N = 100000
NPC_REAL = 12500
NPC = 12544               # = 98 * 128
NT = 98
NC = 8
F_IN = 128
H1, C1 = 2, 16
OUT = 16
WSEG = 256
ROW = 64                  # bf16 elements per node row (128B); pair row = 256B
PAD_AS = -1.0e4
NROWS = NC * NPC // 2     # 50176 pair rows
WIN_A = (0, 32768)        # pair-row range of window A (positions 0..65535)
WIN_B = (17408, 50176)    # pair-row range of window B (positions 34816..)
OVL_LO = 34816            # positions in [OVL_LO, 65536) may go to A or B
OVL_HI = 65536
CHUNK = 36                # max slot columns per dma_gather call (4608 descs)


def _preprocess(x, edge_index, index, W1, att_src1, att_dst1, b1,
                W2, att_src2, att_dst2, b2):
    f32 = np.float32
    src = np.asarray(edge_index[0], dtype=np.int64)
    dst = np.asarray(edge_index[1], dtype=np.int64)
    loops = np.arange(N, dtype=np.int64)
    src = np.concatenate([src, loops]).astype(np.int64)
    dst = np.concatenate([dst, loops]).astype(np.int64)

    owner_dst = dst // NPC_REAL
    ldst = dst - owner_dst * NPC_REAL

    # per-core relabel: degree-sorted descending so each tile's 128 lanes have
    # near-equal degree (minimises ELLPACK padding via max-over-lanes)
    counts_deg = np.bincount(owner_dst * NPC + ldst, minlength=NC * NPC)
    counts_deg = counts_deg.reshape(NC, NPC)
    orders = np.zeros((NC, NPC), dtype=np.int64)
    for c in range(NC):
        orders[c] = np.argsort(-counts_deg[c], kind="stable")
    inv_orders = np.argsort(orders, axis=1)
    pos = np.zeros(N, dtype=np.int64)
    ar = np.arange(NPC_REAL)
    for c in range(NC):
        pos[c * NPC_REAL + ar] = c * NPC + inv_orders[c][ar]

    spos = pos[src]
    new_ldst = inv_orders[owner_dst, ldst]
    nid = owner_dst * NPC + new_ldst

    # window assignment: A = positions < 65536, B = positions >= 34816;
    # overlap edges balance windows. Per tile, pick the cut target T that
    # minimises S_A(T)+S_B(T) where S_w = max over all cores' lanes.
    a_only = spos < OVL_LO
    b_only = spos >= OVL_HI
    deg = np.bincount(nid, minlength=NC * NPC)
    cntA_min = np.bincount(nid[a_only], minlength=NC * NPC)
    cntB_only = np.bincount(nid[b_only], minlength=NC * NPC)
    flex = deg - cntA_min - cntB_only
    tA = np.clip((deg + 1) // 2, cntA_min, cntA_min + flex)
    nA = tA
    nB = deg - tA

    # per-edge window: overlap edges of node n: first (tA - cntA_min) of them
    # (in src-position order) go to A, rest to B.
    eorder = np.lexsort((spos, nid))
    s_nid = nid[eorder]
    s_spos = spos[eorder]
    run_starts = np.zeros(NC * NPC + 1, dtype=np.int64)
    np.cumsum(deg, out=run_starts[1:])
    # rank of each edge within its node's overlap group
    in_ovl = (s_spos >= OVL_LO) & (s_spos < OVL_HI)
    # prefix count of overlap edges within each node's run
    ovl_cum = np.cumsum(in_ovl)
    node_ovl_base = np.concatenate([[0], ovl_cum])[run_starts[s_nid]]
    ovl_rank = ovl_cum - 1 - node_ovl_base  # valid where in_ovl
    takeA = tA - cntA_min
    s_winB = np.where(in_ovl, ovl_rank >= takeA[s_nid], s_spos >= OVL_HI)

    # slot columns per tile: [A slots | B slots]; per-lane fill in src order
    nAB = np.stack([nA, nB], 1).reshape(NC, NPC, 2)
    S = nAB.reshape(NC, NT, 128, 2).max(axis=(0, 2)).astype(np.int64)  # [NT,2]
    WT = S.sum(axis=1)

    # per-call chunking: window A cols [0,S[t,0]) split into <=CHUNK pieces,
    # then window B cols [S[t,0], WT)
    calls = []  # (tile, col0, ncols, window)
    for t in range(NT):
        for w in range(2):
            base = 0 if w == 0 else int(S[t, 0])
            sw = int(S[t, w])
            c0 = 0
            while c0 < sw:
                cn = min(CHUNK, sw - c0)
                calls.append((t, base + c0, cn, w))
                c0 += cn

    idxw_off = [0]
    for (_t, _c0, cn, _w) in calls:
        idxw_off.append(idxw_off[-1] + 8 * cn)
    IDXW = idxw_off[-1]

    # poison pad rows: every core's local ranks 12500.. are zero nodes whose
    # a_src gets PAD_AS added on-device. Pick pad pair-rows inside each
    # window's index range: core 2 spares (positions 37588..) for A, core 7
    # spares (positions 100308..) for B.
    PADROW_A = (2 * NPC + 12500) // 2          # 18794, window A idx 18794
    PADROW_B = (7 * NPC + 12500) // 2          # 50154, window B idx 32746
    padidx = (PADROW_A - WIN_A[0], PADROW_B - WIN_B[0])
    assert 0 <= padidx[0] < 32768 and 0 <= padidx[1] < 32768

    # build per-core gidx + penalties
    gidx = np.zeros((NC, 128, IDXW), dtype=np.int16)
    totWT = int(WT.sum())
    wt_off = np.concatenate([[0], np.cumsum(WT)]).astype(np.int64)
    penE = np.full((NC, 128, totWT), PAD_AS, dtype=np.float32)
    penO = np.full((NC, 128, totWT), PAD_AS, dtype=np.float32)

    # per (core, tile) slot tables, filled from sorted edge runs
    for c in range(NC):
        nodes_all = c * NPC + np.arange(NPC)
        r0 = run_starts[nodes_all]
        d_all = deg[nodes_all]
        nA_all = nA[nodes_all]
        for t in range(NT):
            sA, sB = int(S[t, 0]), int(S[t, 1])
            wt = sA + sB
            nodes = c * NPC + t * 128 + np.arange(128)
            flat = np.full((wt, 128), -1, dtype=np.int64)  # filled below
            par = np.zeros((wt, 128), dtype=bool)
            used = np.zeros((wt, 128), dtype=bool)
            for lane in range(128):
                n = t * 128 + lane
                d = int(d_all[n])
                if d == 0:
                    continue
                e0 = r0[n]
                rows = s_spos[e0:e0 + d] >> 1
                winB = s_winB[e0:e0 + d]
                pr = (s_spos[e0:e0 + d] & 1).astype(bool)
                ia = np.nonzero(~winB)[0]
                ib = np.nonzero(winB)[0]
                assert len(ia) <= sA and len(ib) <= sB, (len(ia), sA, len(ib), sB)
                flat[:len(ia), lane] = rows[ia] - WIN_A[0]
                par[:len(ia), lane] = pr[ia]
                used[:len(ia), lane] = True
                flat[sA:sA + len(ib), lane] = rows[ib] - WIN_B[0]
                par[sA:sA + len(ib), lane] = pr[ib]
                used[sA:sA + len(ib), lane] = True
            flat[:sA][flat[:sA] < 0] = padidx[0]
            flat[sA:][flat[sA:] < 0] = padidx[1]
            # penalties: used even slot -> penE 0; used odd -> penO 0
            pE = np.where(used & ~par, 0.0, PAD_AS).astype(np.float32)
            pO = np.where(used & par, 0.0, PAD_AS).astype(np.float32)
            penE[c, :, wt_off[t]:wt_off[t + 1]] = pE.T
            penO[c, :, wt_off[t]:wt_off[t + 1]] = pO.T
            assert flat.max() < 32768 and flat.min() >= 0
            # emit per-call index blocks
            for ci, (tt, col0, cn, _w) in enumerate(calls):
                if tt != t:
                    continue
                blk = flat[col0:col0 + cn]
                w16 = blk.reshape(-1, 16).T.astype(np.int16)
                gidx[c, :, idxw_off[ci]:idxw_off[ci + 1]] = np.tile(w16, (8, 1))

    bf16 = ml_dtypes.bfloat16
    W1 = np.asarray(W1, f32); W2 = np.asarray(W2, f32)
    as1 = np.asarray(att_src1, f32); ad1 = np.asarray(att_dst1, f32)
    as2 = np.asarray(att_src2, f32); ad2 = np.asarray(att_dst2, f32)
    vs1 = np.stack([W1[:, h * C1:(h + 1) * C1] @ as1[h] for h in range(H1)], 1)
    vd1 = np.stack([W1[:, h * C1:(h + 1) * C1] @ ad1[h] for h in range(H1)], 1)
    wcat1 = np.concatenate([W1, vs1, vd1], axis=1).astype(bf16)   # [128, 36]
    vs2 = (W2 @ as2[0])[:, None]
    vd2 = (W2 @ ad2[0])[:, None]
    wcat2 = np.concatenate([W2, vs2, vd2], axis=1).astype(bf16)   # [32, 18]

    x = np.asarray(x, f32)
    xT = np.zeros((NC, F_IN, NPC), dtype=bf16)
    glb = np.zeros((NC, NPC), dtype=np.int64)
    real = np.zeros((NC, NPC), dtype=bool)
    for c in range(NC):
        ol = orders[c]
        is_real = ol < NPC_REAL
        g = np.where(is_real, c * NPC_REAL + np.minimum(ol, NPC_REAL - 1), 0)
        xT[c] = np.where(is_real[:, None], x[g], 0.0).T.astype(bf16)
        glb[c] = g
        real[c] = is_real

    index = np.asarray(index, np.int64)
    seg = np.zeros((NC, NPC), dtype=np.int64)
    g0 = np.zeros(NC, dtype=np.int64)
    for c in range(NC):
        seg[c] = np.where(real[c], index[glb[c]], 0)
        s = seg[c][real[c]]
        g0[c] = s.min()
        assert s.max() - s.min() < WSEG, "segment window exceeds WSEG"
    f8 = ml_dtypes.float8_e4m3
    ohf = np.zeros((NC, NT * 128, WSEG), dtype=f8)
    oht = np.zeros((NC, NT * 128, WSEG), dtype=f8)
    for c in range(NC):
        for t in range(NT):
            sl = seg[c, t * 128:(t + 1) * 128] - g0[c]
            m = real[c, t * 128:(t + 1) * 128]
            oh = np.zeros((128, WSEG), dtype=np.float32)
            oh[np.arange(128)[m], sl[m]] = 1.0
            ohf[c, t * 128:(t + 1) * 128] = oh.astype(f8)
            ohtk = np.concatenate([oh[:, :128].T, oh[:, 128:].T], axis=1)
            oht[c, t * 128:(t + 1) * 128] = ohtk.astype(f8)

    padfix = np.zeros((128, 3), dtype=f32)
    padfix[84:128, :] = PAD_AS

    sidx = np.zeros((NC, 128, 2), dtype=np.int32)
    for c in range(NC):
        for k in range(2):
            sidx[c, :, k] = g0[c] + k * 128 + np.arange(128)

    b1t = np.tile(np.asarray(b1, f32)[None, :], (128, 1)).astype(f32)
    b2t = np.tile(np.asarray(b2, f32)[None, :], (128, 1)).astype(f32)

    per_core = [{
        "xT": np.ascontiguousarray(xT[c]),
        "wcat1": wcat1, "wcat2": wcat2, "b1t": b1t, "b2t": b2t,
        "gidx": np.ascontiguousarray(gidx[c]),
        "penE": np.ascontiguousarray(penE[c].astype(bf16)),
        "penO": np.ascontiguousarray(penO[c].astype(bf16)),
        "padfix": padfix,
        "ohf": np.ascontiguousarray(ohf[c]),
        "oht": np.ascontiguousarray(oht[c]),
        "sidx": np.ascontiguousarray(sidx[c]),
    } for c in range(NC)]
    shared = {"S": S, "WT": WT, "calls": calls, "idxw_off": idxw_off,
              "IDXW": IDXW, "wt_off": wt_off, "totWT": totWT}
    asm = {"glb": glb, "real": real}
    return shared, per_core, asm


def _build(shared):
    import concourse.bass as bass
    import concourse.bacc as bacc
    import concourse.tile as tile
    from concourse import mybir, library_config
    from concourse.masks import make_identity

    S = shared["S"]; calls = shared["calls"]
    idxw_off = shared["idxw_off"]; IDXW = shared["IDXW"]
    wt_off = shared["wt_off"]; totWT = shared["totWT"]; WT = shared["WT"]
    f32 = mybir.dt.float32
    bf16 = mybir.dt.bfloat16
    f8 = mybir.dt.float8e4
    i16 = mybir.dt.int16
    AL = mybir.AluOpType
    EXP = mybir.ActivationFunctionType.Exp
    PRELU = mybir.ActivationFunctionType.Prelu
    IOA = bass.IndirectOffsetOnAxis

    nc = bacc.Bacc("TRN2", target_bir_lowering=False, debug=False,
                   num_devices=NC, num_swdge_queues=4)

    xT_ext = nc.dram_tensor("xT", [F_IN, NPC], bf16, kind="ExternalInput")
    wcat1_ext = nc.dram_tensor("wcat1", [F_IN, 36], bf16, kind="ExternalInput")
    wcat2_ext = nc.dram_tensor("wcat2", [32, 18], bf16, kind="ExternalInput")
    b1_ext = nc.dram_tensor("b1t", [128, 32], f32, kind="ExternalInput")
    b2_ext = nc.dram_tensor("b2t", [128, 16], f32, kind="ExternalInput")
    gidx_ext = nc.dram_tensor("gidx", [128, IDXW], i16, kind="ExternalInput")
    penE_ext = nc.dram_tensor("penE", [128, totWT], bf16, kind="ExternalInput")
    penO_ext = nc.dram_tensor("penO", [128, totWT], bf16, kind="ExternalInput")
    ohf_ext = nc.dram_tensor("ohf", [NT * 128, WSEG], f8, kind="ExternalInput")
    oht_ext = nc.dram_tensor("oht", [NT * 128, WSEG], f8, kind="ExternalInput")
    sidx_ext = nc.dram_tensor("sidx", [128, 2], mybir.dt.int32, kind="ExternalInput")
    padfix_ext = nc.dram_tensor("padfix", [128, 3], f32, kind="ExternalInput")
    out_ext = nc.dram_tensor("out", [NPC, OUT], f32, kind="ExternalOutput")

    with tile.TileContext(nc) as tc:
        with (
            tc.tile_pool(name="dram", bufs=1, space="DRAM") as dr,
            tc.tile_pool(name="const", bufs=1) as cpool,
            tc.tile_pool(name="sbuf", bufs=4) as sb,
            tc.tile_pool(name="big", bufs=2) as bp,
            tc.tile_pool(name="gat", bufs=5) as gp,
            tc.tile_pool(name="gip", bufs=8) as gip,
            tc.tile_pool(name="psum", bufs=2, space="PSUM") as pp,
            tc.tile_pool(name="psum_seg", bufs=1, space="PSUM") as pseg,
            tc.tile_pool(name="res", bufs=1) as rp,
        ):
            tab1_loc = dr.tile([NPC, ROW], bf16, name="tab1_loc")
            tab2_loc = dr.tile([NPC, ROW], bf16, name="tab2_loc")
            tab1_full = dr.tile([NC * NPC, ROW], bf16, name="tab1_full",
                                addr_space="Shared")
            tab2_full = dr.tile([NC * NPC, ROW], bf16, name="tab2_full",
                                addr_space="Shared")
            s_loc = dr.tile([1280, OUT], f32, name="s_loc")
            s_red = dr.tile([1280, OUT], f32, name="s_red", addr_space="Shared")

            tab1v = tab1_full[:].rearrange("(r two) c -> r (two c)", two=2)
            tab2v = tab2_full[:].rearrange("(r two) c -> r (two c)", two=2)

            nc.gpsimd.load_library(library_config.mlp)

            ident = cpool.tile([128, 128], f32, name="ident")
            make_identity(nc, ident[:])
            wc1 = cpool.tile([F_IN, 36], bf16, name="wc1")
            nc.sync.dma_start(out=wc1[:], in_=wcat1_ext[:, :])
            wc2 = cpool.tile([32, 18], bf16, name="wc2")
            nc.sync.dma_start(out=wc2[:], in_=wcat2_ext[:, :])
            b1s = cpool.tile([128, 32], f32, name="b1s")
            nc.sync.dma_start(out=b1s[:], in_=b1_ext[:, :])
            b2s = cpool.tile([128, 16], f32, name="b2s")
            nc.sync.dma_start(out=b2s[:], in_=b2_ext[:, :])
            pfx = cpool.tile([128, 3], f32, name="pfx")
            nc.sync.dma_start(out=pfx[:], in_=padfix_ext[:, :])

            ad1_all = rp.tile([128, NT * 2], f32, name="ad1_all")
            ad2_all = rp.tile([128, NT], f32, name="ad2_all")
            e_all = rp.tile([128, NT * OUT], f32, name="e_all")
            ebf_all = rp.tile([128, NT * OUT], bf16, name="ebf_all")
            x2_all = rp.tile([128, NT * 32], f32, name="x2_all")

            calls_by_tile = {}
            for ci, (t, col0, cn, w) in enumerate(calls):
                calls_by_tile.setdefault(t, []).append((ci, col0, cn, w))
            qctr = [0]

            def gather_tile(t, tabv, lname):
                wt = int(WT[t])
                gq = gp.tile([128, wt, 2 * ROW], bf16, name=f"g{lname}_{t}",
                             tag="gq")
                gi = gip.tile([128, 8 * wt], i16, name=f"gi{lname}_{t}", tag="gi")
                # per-call index blocks are packed call-major in gidx_ext
                cts = calls_by_tile[t]
                i0 = idxw_off[cts[0][0]]
                i1 = idxw_off[cts[-1][0] + 1]
                nc.sync.dma_start(out=gi[:, 0:(i1 - i0)],
                                  in_=gidx_ext[:, i0:i1])
                for (ci, col0, cn, w) in cts:
                    lo = WIN_A[0] if w == 0 else WIN_B[0]
                    nidx = 128 * cn
                    nc.gpsimd.dma_gather(
                        gq[:, col0:col0 + cn, :],
                        tabv[lo:lo + 32768, :],
                        gi[:, idxw_off[ci] - i0:idxw_off[ci + 1] - i0],
                        nidx, nidx, 2 * ROW, queue_num=qctr[0] % 4,
                        single_packet=False)
                    qctr[0] += 1
                return gq

            # ---- phase 0: layer-1 node rows -------------------------------
            for t in range(NT):
                xt = sb.tile([128, 128], bf16, name=f"xt{t}", tag="xt")
                nc.sync.dma_start(out=xt[:], in_=xT_ext[:, t * 128:(t + 1) * 128])
                hp = pp.tile([128, 36], f32, name=f"hp{t}", tag="hp")
                nc.tensor.matmul(out=hp[:], lhsT=xt[:], rhs=wc1[:],
                                 start=True, stop=True)
                if t == NT - 1:
                    nc.vector.tensor_tensor(out=hp[:, 32:34], in0=hp[:, 32:34],
                                            in1=pfx[:, 0:2], op=AL.add)
                nc.vector.tensor_copy(out=ad1_all[:, 2 * t:2 * t + 2],
                                      in_=hp[:, 34:36])
                hs = sb.tile([128, ROW], bf16, name=f"hs{t}", tag="hs")
                nc.vector.tensor_copy(out=hs[:, 0:34], in_=hp[:, 0:34])
                nc.vector.memset(hs[:, 34:64], 0.0)
                nc.sync.dma_start(out=tab1_loc[t * 128:(t + 1) * 128, :],
                                  in_=hs[:])

            nc.gpsimd.collective_compute(
                "AllGather", AL.bypass, replica_groups=[list(range(NC))],
                ins=[tab1_loc.opt()], outs=[tab1_full.opt()])

            # ---- phase 1: layer-1 aggregation -----------------------------
            for t in range(NT):
                wt = int(WT[t])
                gq = gather_tile(t, tab1v, "1")
                pE = sb.tile([128, wt], bf16, name=f"pE1_{t}", tag="pE")
                pO = sb.tile([128, wt], bf16, name=f"pO1_{t}", tag="pO")
                nc.sync.dma_start(out=pE[:], in_=penE_ext[:, wt_off[t]:wt_off[t + 1]])
                nc.sync.dma_start(out=pO[:], in_=penO_ext[:, wt_off[t]:wt_off[t + 1]])
                z = bp.tile([128, 2 * wt, 2], f32, name=f"z1_{t}", tag="z")
                nc.vector.tensor_tensor(
                    out=z[:, 0:wt, :], in0=gq[:, :, 32:34],
                    in1=pE[:, :, None].to_broadcast([128, wt, 2]), op=AL.add)
                nc.vector.tensor_tensor(
                    out=z[:, wt:, :], in0=gq[:, :, 96:98],
                    in1=pO[:, :, None].to_broadcast([128, wt, 2]), op=AL.add)
                adc = ad1_all[:, 2 * t:2 * t + 2]
                nc.vector.tensor_tensor(
                    out=z[:], in0=z[:],
                    in1=adc[:, None, :].to_broadcast([128, 2 * wt, 2]),
                    op=AL.add)
                # exp(leaky_relu(z)) == max(exp(z), exp(0.2*z))
                e1 = bp.tile([128, 2 * wt, 2], f32, name=f"e1a_{t}", tag="e1")
                nc.scalar.activation(out=e1[:], in_=z[:], func=EXP)
                e2 = bp.tile([128, 2 * wt, 2], f32, name=f"e2a_{t}", tag="e2")
                nc.scalar.activation(out=e2[:], in_=z[:], func=EXP, scale=0.2)
                ex = bp.tile([128, 2 * wt, 2], f32, name=f"ex1_{t}", tag="ex")
                nc.vector.tensor_tensor(out=ex[:], in0=e1[:], in1=e2[:], op=AL.max)
                exb = bp.tile([128, 2 * wt, 2], bf16, name=f"exb1_{t}", tag="exb")
                nc.vector.tensor_copy(out=exb[:], in_=ex[:])
                msg = bp.tile([128, 2 * wt, 32], f32, name=f"msg1_{t}", tag="msg")
                for h in range(2):
                    nc.vector.tensor_tensor(
                        out=msg[:, 0:wt, 16 * h:16 * h + 16],
                        in0=gq[:, :, 16 * h:16 * h + 16],
                        in1=exb[:, 0:wt, h:h + 1].to_broadcast([128, wt, 16]),
                        op=AL.mult)
                    nc.vector.tensor_tensor(
                        out=msg[:, wt:, 16 * h:16 * h + 16],
                        in0=gq[:, :, 64 + 16 * h:80 + 16 * h],
                        in1=exb[:, wt:, h:h + 1].to_broadcast([128, wt, 16]),
                        op=AL.mult)
                num = sb.tile([128, 32], f32, name=f"num1_{t}", tag="num")
                nc.vector.reduce_sum(out=num[:],
                                     in_=msg[:].rearrange("p w e -> p e w"),
                                     axis=mybir.AxisListType.X)
                den = sb.tile([128, 2], f32, name=f"den1_{t}", tag="den")
                nc.vector.reduce_sum(out=den[:],
                                     in_=ex[:].rearrange("p w e -> p e w"),
                                     axis=mybir.AxisListType.X)
                nc.vector.tensor_scalar_max(out=den[:], in0=den[:], scalar1=1e-30)
                rcp = sb.tile([128, 2], f32, name=f"rcp1_{t}", tag="rcp")
                nc.vector.reciprocal(out=rcp[:], in_=den[:])
                x2 = sb.tile([128, 32], f32, name=f"x2_{t}", tag="x2")
                for h in range(2):
                    nc.vector.tensor_scalar_mul(
                        out=x2[:, 16 * h:16 * h + 16],
                        in0=num[:, 16 * h:16 * h + 16],
                        scalar1=rcp[:, h:h + 1])
                nc.vector.tensor_tensor(out=x2[:], in0=x2[:], in1=b1s[:], op=AL.add)
                nc.vector.tensor_scalar_max(out=x2_all[:, 32 * t:32 * (t + 1)],
                                            in0=x2[:], scalar1=0.0)

            # ---- phase 2: layer-2 node rows -------------------------------
            for t in range(NT):
                x2tp = pp.tile([32, 128], f32, name=f"x2tp{t}", tag="hp")
                nc.tensor.transpose(out=x2tp[:],
                                    in_=x2_all[:, 32 * t:32 * (t + 1)],
                                    identity=ident[:])
                x2ts = sb.tile([32, 128], bf16, name=f"x2ts{t}", tag="x2ts")
                nc.vector.tensor_copy(out=x2ts[:], in_=x2tp[:])
                h2p = pp.tile([128, 18], f32, name=f"h2p{t}", tag="dp")
                nc.tensor.matmul(out=h2p[:], lhsT=x2ts[:], rhs=wc2[:],
                                 start=True, stop=True)
                if t == NT - 1:
                    nc.vector.tensor_tensor(out=h2p[:, 16:17], in0=h2p[:, 16:17],
                                            in1=pfx[:, 2:3], op=AL.add)
                nc.vector.tensor_copy(out=ad2_all[:, t:t + 1], in_=h2p[:, 17:18])
                h2s = sb.tile([128, ROW], bf16, name=f"h2s{t}", tag="hs")
                nc.vector.tensor_copy(out=h2s[:, 0:17], in_=h2p[:, 0:17])
                nc.vector.memset(h2s[:, 17:64], 0.0)
                nc.sync.dma_start(out=tab2_loc[t * 128:(t + 1) * 128, :],
                                  in_=h2s[:])

            nc.gpsimd.collective_compute(
                "AllGather", AL.bypass, replica_groups=[list(range(NC))],
                ins=[tab2_loc.opt()], outs=[tab2_full.opt()])

            # ---- phase 3: layer-2 aggregation + exp + segment partials ----
            sp = [pseg.tile([128, OUT], f32, name=f"segp{k}") for k in range(2)]
            for t in range(NT):
                wt = int(WT[t])
                gq = gather_tile(t, tab2v, "2")
                pE = sb.tile([128, wt], bf16, name=f"pE2_{t}", tag="pE")
                pO = sb.tile([128, wt], bf16, name=f"pO2_{t}", tag="pO")
                nc.sync.dma_start(out=pE[:], in_=penE_ext[:, wt_off[t]:wt_off[t + 1]])
                nc.sync.dma_start(out=pO[:], in_=penO_ext[:, wt_off[t]:wt_off[t + 1]])
                z = bp.tile([128, 2 * wt, 1], f32, name=f"z2_{t}", tag="z")
                nc.vector.tensor_tensor(out=z[:, 0:wt, :], in0=gq[:, :, 16:17],
                                        in1=pE[:, :, None], op=AL.add)
                nc.vector.tensor_tensor(out=z[:, wt:, :], in0=gq[:, :, 80:81],
                                        in1=pO[:, :, None], op=AL.add)
                adc = ad2_all[:, t:t + 1]
                nc.vector.tensor_tensor(
                    out=z[:], in0=z[:],
                    in1=adc[:, None, :].to_broadcast([128, 2 * wt, 1]),
                    op=AL.add)
                e1 = bp.tile([128, 2 * wt, 1], f32, name=f"e1b_{t}", tag="e1")
                nc.scalar.activation(out=e1[:], in_=z[:], func=EXP)
                e2 = bp.tile([128, 2 * wt, 1], f32, name=f"e2b_{t}", tag="e2")
                nc.scalar.activation(out=e2[:], in_=z[:], func=EXP, scale=0.2)
                ex = bp.tile([128, 2 * wt, 1], f32, name=f"ex2_{t}", tag="ex")
                nc.vector.tensor_tensor(out=ex[:], in0=e1[:], in1=e2[:], op=AL.max)
                den = sb.tile([128, 1], f32, name=f"den2_{t}", tag="den")
                nc.vector.reduce_sum(out=den[:],
                                     in_=ex[:].rearrange("p w e -> p e w"),
                                     axis=mybir.AxisListType.X)
                exb = bp.tile([128, 2 * wt, 1], bf16, name=f"exb2_{t}", tag="exb")
                nc.vector.tensor_copy(out=exb[:], in_=ex[:])
                msg = bp.tile([128, 2 * wt, 16], f32, name=f"msg2_{t}", tag="msg")
                nc.vector.tensor_tensor(
                    out=msg[:, 0:wt, :], in0=gq[:, :, 0:16],
                    in1=exb[:, 0:wt, :].to_broadcast([128, wt, 16]), op=AL.mult)
                nc.vector.tensor_tensor(
                    out=msg[:, wt:, :], in0=gq[:, :, 64:80],
                    in1=exb[:, wt:, :].to_broadcast([128, wt, 16]), op=AL.mult)
                num = sb.tile([128, OUT], f32, name=f"num2_{t}", tag="num")
                nc.vector.reduce_sum(out=num[:],
                                     in_=msg[:].rearrange("p w e -> p e w"),
                                     axis=mybir.AxisListType.X)
                nc.vector.tensor_scalar_max(out=den[:], in0=den[:], scalar1=1e-30)
                rcp = sb.tile([128, 1], f32, name=f"rcp2_{t}", tag="rcp")
                nc.vector.reciprocal(out=rcp[:], in_=den[:])
                o2 = sb.tile([128, OUT], f32, name=f"o2_{t}", tag="o2")
                nc.vector.tensor_scalar_mul(out=o2[:], in0=num[:],
                                            scalar1=rcp[:, 0:1])
                nc.vector.tensor_tensor(out=o2[:], in0=o2[:], in1=b2s[:], op=AL.add)
                nc.scalar.activation(out=e_all[:, OUT * t:OUT * (t + 1)],
                                     in_=o2[:], func=EXP)
                nc.vector.tensor_copy(out=ebf_all[:, OUT * t:OUT * (t + 1)],
                                      in_=e_all[:, OUT * t:OUT * (t + 1)])
                ohf_t = sb.tile([128, WSEG], f8, name=f"ohf{t}", tag="ohf")
                nc.sync.dma_start(out=ohf_t[:],
                                  in_=ohf_ext[t * 128:(t + 1) * 128, :])
                for k in range(2):
                    nc.tensor.matmul(out=sp[k][:],
                                     lhsT=ohf_t[:, k * 128:(k + 1) * 128],
                                     rhs=ebf_all[:, OUT * t:OUT * (t + 1)],
                                     start=(t == 0), stop=(t == NT - 1))

            # ---- phase 4: combine segment sums across cores ---------------
            zt = sb.tile([128, 160], f32, name="zt")
            nc.vector.memset(zt[:], 0.0)
            nc.sync.dma_start(
                out=s_loc.rearrange("(c p) f -> p c f", p=128),
                in_=zt[:].rearrange("p (c f) -> p c f", c=10))
            sxi = sb.tile([128, 2], mybir.dt.int32, name="sxi")
            nc.sync.dma_start(out=sxi[:], in_=sidx_ext[:, :])
            for k in range(2):
                spc = sb.tile([128, OUT], f32, name=f"spc{k}", tag="spc")
                nc.vector.tensor_copy(out=spc[:], in_=sp[k][:])
                nc.gpsimd.indirect_dma_start(
                    out=s_loc[:, :],
                    out_offset=IOA(ap=sxi[:, k:k + 1], axis=0),
                    in_=spc[:], in_offset=None)

            nc.gpsimd.collective_compute(
                "AllReduce", AL.add, replica_groups=[list(range(NC))],
                ins=[s_loc.opt()], outs=[s_red.opt()])

            sw = []
            for k in range(2):
                swf = sb.tile([128, OUT], f32, name=f"swf{k}", tag="swf")
                nc.gpsimd.indirect_dma_start(
                    out=swf[:], out_offset=None,
                    in_=s_red[:, :],
                    in_offset=IOA(ap=sxi[:, k:k + 1], axis=0))
                swb = rp.tile([128, OUT], bf16, name=f"sw{k}")
                nc.vector.tensor_copy(out=swb[:], in_=swf[:])
                sw.append(swb)

            # ---- phase 5: divide, write out -------------------------------
            for t in range(NT):
                oht_t = sb.tile([128, WSEG], f8, name=f"oht{t}", tag="oht")
                nc.sync.dma_start(out=oht_t[:],
                                  in_=oht_ext[t * 128:(t + 1) * 128, :])
                dp = pp.tile([128, OUT], f32, name=f"dp{t}", tag="dp")
                for k in range(2):
                    nc.tensor.matmul(out=dp[:],
                                     lhsT=oht_t[:, k * 128:(k + 1) * 128],
                                     rhs=sw[k][:], start=(k == 0), stop=(k == 1))
                dd = sb.tile([128, OUT], f32, name=f"dd{t}", tag="dd")
                nc.vector.tensor_scalar_max(out=dd[:], in0=dp[:], scalar1=1e-30)
                nc.vector.reciprocal(out=dd[:], in_=dd[:])
                fo = sb.tile([128, OUT], f32, name=f"fo{t}", tag="fo")
                nc.vector.tensor_tensor(out=fo[:],
                                        in0=e_all[:, OUT * t:OUT * (t + 1)],
                                        in1=dd[:], op=AL.mult)
                nc.sync.dma_start(out=out_ext[t * 128:(t + 1) * 128, :], in_=fo[:])

    nc.compile()
    return nc


def kernel_impl(inputs, trace=False, tmpdir=None):
    from concourse.bass_utils import run_bass_kernel_spmd
    shared, per_core, asm = _preprocess(**inputs)
    nc = _build(shared)
    res = run_bass_kernel_spmd(nc, per_core, core_ids=list(range(NC)),
                               trace=trace, tmpdir=tmpdir)
    out = np.zeros((N, OUT), dtype=np.float32)
    for c in range(NC):
        o = np.asarray(res.results[c]["out"])
        m = asm["real"][c]
        out[asm["glb"][c][m]] = o[m]
    return out, res


def kernel(**inputs):
    out, _ = kernel_impl(inputs, trace=False)
    return out

